# revision 7
# baseline (speedup 1.0000x reference)
"""Local (sliding-window) attention on 8 Trainium2 NeuronCores.

Problem: B=2, T=2048, H=8, E=64, local_context C=128.
Query i attends keys [i-64, i+64) (clipped to [0, T)).

Sharding: the 16 (b,h) pairs are split 2-per-core (pure data parallel,
no halo needed).

Per-core kernel (2 head-pairs, T=2048):
  - Host pre-transposes Q,K to [E, T] bf16 and packs the two pairs onto
    the 128 SBUF partitions (pair0 -> partitions 0:64, pair1 -> 64:128),
    so no on-chip transposes are needed and DMAs are full-width.
  - Key chunks C_j = keys [128j+64, 128j+192), j = -1..15 (edges clamped
    to [0,128) / [1920,2048)). Query tile i in [0,16) covers queries
    [128i, 128i+128) and needs chunks C_{i-1} (A-part) and C_i (B-part).
  - Scores computed transposed: S^T[key, query] = matmul(lhsT=K^T chunk,
    rhs=Q^T two query tiles) -> [128, 256] block; 4 chunks share one
    [128, 1024] PSUM tile so ACT's 352-cycle fixed cost per ACTIVATE is
    amortized: one exp(x/8) per group, PSUM -> SBUF bf16.
  - Band masking is multiplicative-0/1 in bf16 AFTER exp (DVE 2x mode),
    one big tensor_mul per group.
  - AV: out[query, e] accumulates matmul(lhsT=expS^T part, rhs=V chunk)
    over the A and B parts; expS^T is already the correct stationary
    orientation (no P transpose). V chunks carry a 65th column of ones
    so col 64 of the PSUM accumulator is the softmax denominator.
  - 4 query tiles share one [128, 260] PSUM accumulator; one batched
    reciprocal per 4 tiles, one tensor_scalar_mul per tile -> f32 out.
"""

import numpy as np

B, T, H, E = 2, 2048, 8, 64
C = 128
HALF = C // 2  # 64
NT = T // 128  # 16 query tiles per head
NCORES = 8
PAIRS_PER_CORE = (B * H) // NCORES  # 2

_cache = {}


def _build():
    import concourse.bass as bass
    import concourse.mybir as mybir
    import concourse.tile as tile
    from concourse import bacc

    f32 = mybir.dt.float32
    bf16 = mybir.dt.bfloat16
    AF = mybir.ActivationFunctionType
    ALU = mybir.AluOpType

    nc = bacc.Bacc("TRN2", target_bir_lowering=False, debug=False)
    qt_d = nc.dram_tensor("qt", (128, T), bf16, kind="ExternalInput").ap()
    kt_d = nc.dram_tensor("kt", (128, T), bf16, kind="ExternalInput").ap()
    v_d = nc.dram_tensor("v", (PAIRS_PER_CORE, T, E), bf16, kind="ExternalInput").ap()
    o_d = nc.dram_tensor("o", (PAIRS_PER_CORE, T, E), f32, kind="ExternalOutput").ap()

    with tile.TileContext(nc) as tc:
        with (
            tc.tile_pool(name="const", bufs=1) as cpool,
            tc.tile_pool(name="io", bufs=1) as iopool,
            tc.tile_pool(name="es", bufs=6) as espool,
            tc.tile_pool(name="small", bufs=4) as spool,
            tc.tile_pool(name="ps_s", bufs=2, space="PSUM") as ps_s,
            tc.tile_pool(name="ps_o", bufs=3, space="PSUM") as ps_o,
        ):
            # ---- resident data tiles ----
            qt_sb = iopool.tile([128, T], bf16, tag="qt")
            kt_sb = iopool.tile([128, T], bf16, tag="kt")
            v_sb = [iopool.tile([128, NT + 1, E + 1], bf16, tag=f"v{p}",
                                name=f"v_sb{p}") for p in range(PAIRS_PER_CORE)]
            o_sb = [iopool.tile([128, NT, E], f32, tag=f"o{p}", name=f"o_sb{p}")
                    for p in range(PAIRS_PER_CORE)]

            # input DMAs up front, issue spread across SP / ACT / POOL
            nc.sync.dma_start(kt_sb[:], kt_d[:])
            nc.scalar.dma_start(qt_sb[:], qt_d[:])
            # ones column for the softmax denominator (DVE; tiny)
            for p in range(PAIRS_PER_CORE):
                nc.vector.memset(v_sb[p][:, :, E:E + 1], 1.0)
            # v slots: slot k holds keys [128k-64, 128k+64), edges clamped
            v_mid = [
                v_d[p, HALF:HALF + (NT - 1) * 128, :].rearrange(
                    "(k p) e -> p k e", p=128)
                for p in range(PAIRS_PER_CORE)
            ]
            for p in range(PAIRS_PER_CORE):
                nc.gpsimd.dma_start(v_sb[p][:, 0, :E], v_d[p, 0:128, :])
                nc.gpsimd.dma_start(v_sb[p][:, 1:NT, :E], v_mid[p][:, :, :])
                nc.gpsimd.dma_start(v_sb[p][:, NT, :E], v_d[p, T - 128:T, :])

            # ---- 0/1 band masks in bf16 (built once on POOL) ----
            # interior chunk block [128 keys x 256 queries]: key row p <->
            # global key 128j+64+p; col c <-> query 128j+c; valid iff
            # c-128 <= p <= c-1, i.e. (c-p-1 >= 0) and (p-c+128 >= 0).
            # m_g covers groups of 4 chunk blocks (period 256).
            m_int4 = cpool.tile([128, 1024], bf16, tag="m_int4")
            nc.gpsimd.memset(m_int4[:], 1.0)
            nc.gpsimd.affine_select(
                out=m_int4[:], in_=m_int4[:], compare_op=ALU.is_ge, fill=0.0,
                base=-1, channel_multiplier=-1, pattern=[[0, 4], [1, 256]])
            nc.gpsimd.affine_select(
                out=m_int4[:], in_=m_int4[:], compare_op=ALU.is_ge, fill=0.0,
                base=128, channel_multiplier=1, pattern=[[0, 4], [-1, 256]])

            # group-0 mask [128, 896] for es cols [128:1024]:
            #   cols 0:128 = first-tile A-part mask (keys clamped to [0,128)):
            #     valid iff p <= 63 and p >= c-64
            #   cols 128:896 = 3 interior blocks
            m_g0 = cpool.tile([128, 1024 - 128], bf16, tag="m_g0")
            nc.gpsimd.memset(m_g0[:], 1.0)
            nc.gpsimd.affine_select(
                out=m_g0[:, 0:128], in_=m_g0[:, 0:128], compare_op=ALU.is_ge,
                fill=0.0, base=63, channel_multiplier=-1, pattern=[[0, 128]])
            nc.gpsimd.affine_select(
                out=m_g0[:, 0:128], in_=m_g0[:, 0:128], compare_op=ALU.is_ge,
                fill=0.0, base=64, channel_multiplier=1, pattern=[[-1, 128]])
            nc.gpsimd.affine_select(
                out=m_g0[:, 128:896], in_=m_g0[:, 128:896],
                compare_op=ALU.is_ge, fill=0.0,
                base=-1, channel_multiplier=-1, pattern=[[0, 3], [1, 256]])
            nc.gpsimd.affine_select(
                out=m_g0[:, 128:896], in_=m_g0[:, 128:896],
                compare_op=ALU.is_ge, fill=0.0,
                base=128, channel_multiplier=1, pattern=[[0, 3], [-1, 256]])

            # last-tile B-part mask [128, 128] (keys clamped to [1920,2048)):
            # valid iff p >= 64 and p <= c+63
            m_last = cpool.tile([128, 128], bf16, tag="m_last")
            nc.gpsimd.memset(m_last[:], 1.0)
            nc.gpsimd.affine_select(
                out=m_last[:], in_=m_last[:], compare_op=ALU.is_ge, fill=0.0,
                base=-64, channel_multiplier=1, pattern=[[0, 128]])
            nc.gpsimd.affine_select(
                out=m_last[:], in_=m_last[:], compare_op=ALU.is_ge, fill=0.0,
                base=63, channel_multiplier=-1, pattern=[[1, 128]])

            o_r = [o_d[p].rearrange("(i p) e -> p i e", p=128)
                   for p in range(PAIRS_PER_CORE)]

            # chunk j lives in group g=(j+1)//4 at block b=(j+1)%4.
            # groups 0-3 are [128,1024] (4 blocks), group 4 is [128,256].
            es_groups = [[None] * 5 for _ in range(PAIRS_PER_CORE)]
            po4 = [[None] * 4 for _ in range(PAIRS_PER_CORE)]

            def scores_group(p, g):
                """matmuls + exp + mask for all chunks of group g."""
                pb = 64 * p
                width = 256 if g == 4 else 1024
                ps = ps_s.tile([128, 1024], f32, tag="ps_s", name="ps")
                chunks = range(4 * g - 1, min(4 * g + 3, NT))
                for j in chunks:
                    b = (j + 1) % 4
                    if j == -1:
                        k0, q0, nq, oc = 0, 0, 128, 128
                    elif j == NT - 1:
                        k0, q0, nq, oc = T - 128, T - 128, 128, 0
                    else:
                        k0, q0, nq, oc = 128 * j + HALF, 128 * j, 256, 0
                    nc.tensor.matmul(
                        ps[:, 256 * b + oc: 256 * b + oc + nq],
                        kt_sb[pb:pb + 64, k0:k0 + 128],
                        qt_sb[pb:pb + 64, q0:q0 + nq],
                        start=True, stop=True)
                es = espool.tile([128, 1024], bf16, tag="es", name="es")
                nc.scalar.activation(es[:, :width], ps[:, :width],
                                     AF.Exp, scale=1.0 / np.sqrt(E))
                if g == 0:
                    nc.vector.tensor_mul(es[:, 128:1024], es[:, 128:1024],
                                         m_g0[:])
                elif g == 4:
                    nc.vector.tensor_mul(es[:, 0:128], es[:, 0:128], m_last[:])
                else:
                    nc.vector.tensor_mul(es[:, :], es[:, :], m_int4[:])
                es_groups[p][g] = es

            def es_slice(p, j, lo, hi):
                g, b = (j + 1) // 4, (j + 1) % 4
                return es_groups[p][g][:, 256 * b + lo: 256 * b + hi]

            def av(p, i):
                """accumulate query tile i into its shared po4 accumulator."""
                t, m = i // 4, i % 4
                if m == 0:
                    po4[p][t] = ps_o.tile([128, 4 * (E + 1)], f32,
                                          tag="po4", name="po4")
                po = po4[p][t][:, 65 * m: 65 * m + 65]
                nc.tensor.matmul(po, es_slice(p, i - 1, 128, 256),
                                 v_sb[p][:, i, :], start=True, stop=False)
                nc.tensor.matmul(po, es_slice(p, i, 0, 128),
                                 v_sb[p][:, i + 1, :], start=False, stop=True)

            def norm4(p, t):
                """normalize query tiles 4t..4t+3 -> o_sb."""
                po = po4[p][t].rearrange("p (t c) -> p t c", c=E + 1)
                rec = spool.tile([128, 4], f32, tag="rec", name="rec")
                nc.vector.reciprocal(rec[:], po[:, :, E])
                for m in range(4):
                    nc.vector.tensor_scalar_mul(
                        o_sb[p][:, 4 * t + m, :], po[:, m, 0:E],
                        rec[:, m:m + 1])

            for g in range(5):
                for p in range(PAIRS_PER_CORE):
                    scores_group(p, g)
                # tiles with both chunks available: chunk <= 4g+2
                lo = 0 if g == 0 else 4 * g - 1
                hi = min(4 * g + 3, NT)
                for p in range(PAIRS_PER_CORE):
                    for i in range(lo, hi):
                        av(p, i)
                        if i % 4 == 3:
                            norm4(p, i // 4)
            for p in range(PAIRS_PER_CORE):
                nc.sync.dma_start(o_r[p][:, :, :], o_sb[p][:, :, :])

    nc.compile()
    return nc


def _get_nc():
    if "nc" not in _cache:
        _cache["nc"] = _build()
    return _cache["nc"]


def kernel(query, key, value, local_context):
    import ml_dtypes
    from concourse import bass_utils

    assert int(local_context) == C
    assert query.shape == (B, T, H, E)
    nc = _get_nc()

    bf = ml_dtypes.bfloat16
    # (B,T,H,E) -> (B*H, T, E)
    qh = np.ascontiguousarray(query.transpose(0, 2, 1, 3)).reshape(B * H, T, E)
    kh = np.ascontiguousarray(key.transpose(0, 2, 1, 3)).reshape(B * H, T, E)
    vh = np.ascontiguousarray(value.transpose(0, 2, 1, 3)).reshape(B * H, T, E)

    in_maps = []
    for c in range(NCORES):
        p0 = PAIRS_PER_CORE * c
        qt = np.ascontiguousarray(
            qh[p0:p0 + PAIRS_PER_CORE].transpose(0, 2, 1).reshape(128, T)
        ).astype(bf)
        kt = np.ascontiguousarray(
            kh[p0:p0 + PAIRS_PER_CORE].transpose(0, 2, 1).reshape(128, T)
        ).astype(bf)
        v = np.ascontiguousarray(vh[p0:p0 + PAIRS_PER_CORE]).astype(bf)
        in_maps.append({"qt": qt, "kt": kt, "v": v})

    res = bass_utils.run_bass_kernel_spmd(nc, in_maps, core_ids=list(range(NCORES)))
    _cache["last_results"] = res

    oh = np.empty((B * H, T, E), dtype=np.float32)
    for c in range(NCORES):
        p0 = PAIRS_PER_CORE * c
        oh[p0:p0 + PAIRS_PER_CORE] = res.results[c]["o"]
    out = oh.reshape(B, H, T, E).transpose(0, 2, 1, 3)
    return np.ascontiguousarray(out)
